# revision 9
# baseline (speedup 1.0000x reference)
"""AttentionRouter MoE-routing kernel for 8 Trainium2 NeuronCores.

Strategy: data-parallel over the 8192 tokens (1024 tokens/core), weights
replicated. Per core:
  A: interT = relu(w1.T @ xT + b1)          (transposed activations on chip)
  C: proj   = x @ wp  (+bp), then LayerNorm
  B: logits = inter @ w2 (+b2) -> DRAM scratch
  D: top-8 via DVE max/max_index, top-k softmax, indirect-DMA gather of
     tensor_pool rows, weighted combine
  E/F: concat-transpose + final matmul with wm (+bm)
Host: shard/unshard, pre-transpose xT, pre-split operands to bf16 hi/lo
pairs (exact 3-pass bf16x2 matmuls ~ fp32 accuracy), usage stats +
diversity loss from returned top-k indices/weights (O(P) work).
"""
import os
import sys

sys.path.insert(0, "/opt/trn_rl_repo")

import numpy as np
import ml_dtypes

import concourse.bass as bass
import concourse.bacc as bacc
import concourse.mybir as mybir
import concourse.tile as tile
from concourse.bass_utils import run_bass_kernel_spmd
from concourse.masks import make_identity

F32 = mybir.dt.float32
F32R = mybir.dt.float32r
BF16 = mybir.dt.bfloat16
U32 = mybir.dt.uint32
AF = mybir.ActivationFunctionType
ALU = mybir.AluOpType
AX = mybir.AxisListType

NCORES = 8
TOK = 1024            # tokens per core
NT = TOK // 128       # 8 token tiles per core
H, I, PP, D = 2048, 1024, 4096, 1024
KH, KI = H // 128, I // 128      # 16, 8 contraction chunks
EPS = 1e-5

# precision modes: "split" = bf16x2 3-pass (fp32-grade), "bf16" = single pass
ROUTER_MODE = "split"   # mm1 + mm2 (feeds top-k selection: needs precision)
TAIL_MODE = "split"     # mmp + mmm (output path)


def _mm_acc(nc, psum, lhs_pair, rhs_pair, nk, start):
    """Accumulate lhsT.T @ rhs into psum with bf16x2 3-pass (or single pass).

    lhs_pair/rhs_pair: (hi, lo) tuples of per-chunk slice getters; lo may be
    None for single-pass mode. nk = number of 128-deep contraction chunks.
    """
    lh, ll = lhs_pair
    rh, rl = rhs_pair
    passes = [(lh, rh)]
    if ll is not None:
        passes += [(lh, rl), (ll, rh)]
    n = len(passes) * nk
    i = 0
    for pl, pr in passes:
        for c in range(nk):
            nc.tensor.matmul(psum, pl(c), pr(c),
                             start=(start and i == 0), stop=(i == n - 1))
            i += 1


def _build(inv_temp: float, top_k: int, use_b2: bool, use_bp: bool,
           use_gamma: bool, use_beta: bool, use_bm: bool):
    split_router = ROUTER_MODE == "split"
    split_tail = TAIL_MODE == "split"
    nc = bacc.Bacc("TRN2", target_bir_lowering=False, debug=False,
                   num_devices=NCORES)

    def din(name, shape, dtype=F32):
        return nc.dram_tensor(name, shape, dtype, kind="ExternalInput")

    # per-core inputs (already transposed / split host-side)
    xT_hi = din("xT_hi", [H, TOK], BF16)
    xT_lo = din("xT_lo", [H, TOK], BF16) if (split_router or split_tail) else None
    w1_hi = din("w1_hi", [H, I], BF16)
    w1_lo = din("w1_lo", [H, I], BF16) if split_router else None
    w2_hi = din("w2_hi", [I, PP], BF16)
    w2_lo = din("w2_lo", [I, PP], BF16) if split_router else None
    wp_hi = din("wp_hi", [H, D], BF16)
    wp_lo = din("wp_lo", [H, D], BF16) if split_tail else None
    wm_hi = din("wm_hi", [2 * D, D], BF16)
    wm_lo = din("wm_lo", [2 * D, D], BF16) if split_tail else None
    b1_d = din("b1", [I])
    pool_d = din("tensor_pool", [PP, D])
    b2_d = din("b2", [PP]) if use_b2 else None
    bp_d = din("bp", [D]) if use_bp else None
    gamma_d = din("gamma", [D]) if use_gamma else None
    beta_d = din("beta", [D]) if use_beta else None
    bm_d = din("bm", [D]) if use_bm else None

    out_d = nc.dram_tensor("out_sh", [TOK, D], F32, kind="ExternalOutput")
    tki_d = nc.dram_tensor("tk_idx", [TOK, top_k], U32, kind="ExternalOutput")
    tkw_d = nc.dram_tensor("tk_w", [TOK, top_k], F32, kind="ExternalOutput")
    logit_d = nc.dram_tensor("logit_scratch", [TOK, PP], F32)

    def bcast(dram_vec, n):
        """AP that DMA-replicates a [n] DRAM vector across 128 partitions."""
        a = dram_vec.ap()
        return bass.AP(tensor=a.tensor, offset=a.offset, ap=[[0, 128], [1, n]])

    from contextlib import ExitStack
    with tile.TileContext(nc) as tc, ExitStack() as _es:
        cst = _es.enter_context(tc.tile_pool(name="cst", bufs=1))
        ident = cst.tile([128, 128], BF16)
        make_identity(nc, ident[:])
        b1_t = cst.tile([128, KI], F32)
        nc.sync.dma_start(b1_t[:], b1_d.ap().rearrange("(c p) -> p c", p=128))
        eps_t = cst.tile([128, 1], F32)
        nc.vector.memset(eps_t[:], EPS)
        b2_t = bp_t = gamma_t = beta_t = bm_t = None
        if use_b2:
            b2_t = cst.tile([128, PP], F32)
            nc.sync.dma_start(b2_t[:], bcast(b2_d, PP))
        if use_bp:
            bp_t = cst.tile([128, D], F32)
            nc.sync.dma_start(bp_t[:], bcast(bp_d, D))
        if use_gamma:
            gamma_t = cst.tile([128, D], F32)
            nc.sync.dma_start(gamma_t[:], bcast(gamma_d, D))
        if use_beta:
            beta_t = cst.tile([128, D], F32)
            nc.sync.dma_start(beta_t[:], bcast(beta_d, D))
        if use_bm:
            bm_t = cst.tile([128, D], F32)
            nc.sync.dma_start(bm_t[:], bcast(bm_d, D))

        proj_pool = _es.enter_context(tc.tile_pool(name="proj", bufs=1))
        proj = proj_pool.tile([128, NT, D], F32)          # 32KB/part

        # ---- load xT / interT region ----
        with tc.tile_pool(name="xT", bufs=1) as xTp, \
             tc.tile_pool(name="interT", bufs=1) as iTp:
            xh = xTp.tile([128, KH, TOK], BF16)
            nc.sync.dma_start(xh[:], xT_hi.ap().rearrange("(c p) t -> p c t", p=128))
            xl = None
            if xT_lo is not None:
                xl = xTp.tile([128, KH, TOK], BF16)
                nc.sync.dma_start(xl[:], xT_lo.ap().rearrange("(c p) t -> p c t", p=128))
            ih = iTp.tile([128, KI, TOK], BF16)
            il = iTp.tile([128, KI, TOK], BF16, name="il") if split_router else None

            # ---- phase A: interT = relu(w1.T @ xT + b1) ----
            with tc.tile_pool(name="w1p", bufs=2) as w1p, \
                 tc.tile_pool(name="Aps", bufs=4, space="PSUM") as Aps, \
                 tc.tile_pool(name="Astg", bufs=3) as Astg:
                for mc in range(KI):
                    w1h = w1p.tile([128, KH, 128], BF16, tag="w1h")
                    nc.sync.dma_start(
                        w1h[:], w1_hi.ap()[:, mc * 128:(mc + 1) * 128]
                        .rearrange("(c p) m -> p c m", p=128))
                    w1l = None
                    if split_router:
                        w1l = w1p.tile([128, KH, 128], BF16, tag="w1l")
                        nc.sync.dma_start(
                            w1l[:], w1_lo.ap()[:, mc * 128:(mc + 1) * 128]
                            .rearrange("(c p) m -> p c m", p=128))
                    for sp in range(2):
                        ssl = slice(sp * 512, (sp + 1) * 512)
                        ps = Aps.tile([128, 512], F32, tag="ps")
                        _mm_acc(nc, ps[:],
                                (lambda c: w1h[:, c, :],
                                 (lambda c: w1l[:, c, :]) if split_router else None),
                                (lambda c: xh[:, c, ssl],
                                 (lambda c: xl[:, c, ssl]) if split_router else None),
                                KH, True)
                        stg = Astg.tile([128, 512], F32, tag="stg")
                        nc.scalar.activation(stg[:], ps[:], AF.Relu,
                                             bias=b1_t[:, mc:mc + 1])
                        nc.vector.tensor_copy(ih[:, mc, ssl], stg[:])
                        if split_router:
                            nc.vector.tensor_sub(il[:, mc, ssl], stg[:], ih[:, mc, ssl])

            # ---- phase C: proj = x @ wp (+bp) ----
            with tc.tile_pool(name="wpp", bufs=1) as wpp, \
                 tc.tile_pool(name="Cps", bufs=4, space="PSUM") as Cps:
                wph = wpp.tile([128, KH, D], BF16, tag="wph")
                nc.sync.dma_start(wph[:], wp_hi.ap().rearrange("(c p) m -> p c m", p=128))
                wpl = None
                if split_tail:
                    wpl = wpp.tile([128, KH, D], BF16, tag="wpl")
                    nc.sync.dma_start(wpl[:], wp_lo.ap().rearrange("(c p) m -> p c m", p=128))
                for t in range(NT):
                    tsl = slice(t * 128, (t + 1) * 128)
                    for nn in range(2):
                        nsl = slice(nn * 512, (nn + 1) * 512)
                        ps = Cps.tile([128, 512], F32, tag="ps")
                        _mm_acc(nc, ps[:],
                                (lambda c: xh[:, c, tsl],
                                 (lambda c: xl[:, c, tsl]) if split_tail else None),
                                (lambda c: wph[:, c, nsl],
                                 (lambda c: wpl[:, c, nsl]) if split_tail else None),
                                KH, True)
                        if use_bp:
                            nc.vector.scalar_tensor_tensor(
                                proj[:, t, nsl], ps[:], 1.0, bp_t[:, nsl],
                                op0=ALU.mult, op1=ALU.add)
                        else:
                            nc.scalar.copy(proj[:, t, nsl], ps[:])

            # ---- phase B: logits = inter @ w2 (+b2) -> DRAM ----
            with tc.tile_pool(name="w2p", bufs=2) as w2p, \
                 tc.tile_pool(name="Bps", bufs=4, space="PSUM") as Bps, \
                 tc.tile_pool(name="Bstg", bufs=3) as Bstg:
                for pc in range(PP // 512):
                    psl = slice(pc * 512, (pc + 1) * 512)
                    w2h = w2p.tile([128, KI, 512], BF16, tag="w2h")
                    nc.sync.dma_start(
                        w2h[:], w2_hi.ap()[:, psl].rearrange("(c p) m -> p c m", p=128))
                    w2l = None
                    if split_router:
                        w2l = w2p.tile([128, KI, 512], BF16, tag="w2l")
                        nc.sync.dma_start(
                            w2l[:], w2_lo.ap()[:, psl].rearrange("(c p) m -> p c m", p=128))
                    for t in range(NT):
                        tsl = slice(t * 128, (t + 1) * 128)
                        ps = Bps.tile([128, 512], F32, tag="ps")
                        _mm_acc(nc, ps[:],
                                (lambda c: ih[:, c, tsl],
                                 (lambda c: il[:, c, tsl]) if split_router else None),
                                (lambda c: w2h[:, c, :],
                                 (lambda c: w2l[:, c, :]) if split_router else None),
                                KI, True)
                        stg = Bstg.tile([128, 512], F32, tag="stg")
                        if use_b2:
                            nc.vector.scalar_tensor_tensor(
                                stg[:], ps[:], 1.0, b2_t[:, psl],
                                op0=ALU.mult, op1=ALU.add)
                        else:
                            nc.scalar.copy(stg[:], ps[:])
                        nc.sync.dma_start(logit_d.ap()[t * 128:(t + 1) * 128, psl],
                                          stg[:])

        # ---- phases D/E/F per token tile ----
        with tc.tile_pool(name="wgt", bufs=2) as wgtp, \
             tc.tile_pool(name="wmp", bufs=1) as wmp, \
             tc.tile_pool(name="Dstg", bufs=2) as Dstg, \
             tc.tile_pool(name="Dsml", bufs=4) as Dsml, \
             tc.tile_pool(name="combp", bufs=2) as combp, \
             tc.tile_pool(name="Fps", bufs=2, space="PSUM") as Fps, \
             tc.tile_pool(name="Tps", bufs=2, space="PSUM") as Tps:
            wmh = wmp.tile([128, 2 * KI, D], BF16, tag="wmh")
            nc.sync.dma_start(wmh[:], wm_hi.ap().rearrange("(c p) m -> p c m", p=128))
            wml = None
            if split_tail:
                wml = wmp.tile([128, 2 * KI, D], BF16, tag="wml")
                nc.sync.dma_start(wml[:], wm_lo.ap().rearrange("(c p) m -> p c m", p=128))

            for t in range(NT):
                rsl = slice(t * 128, (t + 1) * 128)
                # --- D: top-k + softmax + gather + weighted combine
                lg = Dstg.tile([128, PP], F32, tag="lg")
                nc.sync.dma_start(lg[:], logit_d.ap()[rsl, :])
                mx8 = Dsml.tile([128, 8], F32, tag="mx8")
                mi8 = Dsml.tile([128, 8], U32, tag="mi8")
                nc.vector.max(out=mx8[:], in_=lg[:])
                nc.vector.max_index(out=mi8[:], in_max=mx8[:], in_values=lg[:])
                vals = Dsml.tile([128, top_k], F32, tag="vals")
                # scaled = clip(logit/temp, -10, 10)
                nc.vector.tensor_scalar(vals[:], mx8[:, :top_k], inv_temp, 10.0,
                                        op0=ALU.mult, op1=ALU.min)
                nc.vector.tensor_scalar_max(vals[:], vals[:], -10.0)
                ex = Dsml.tile([128, top_k], F32, tag="ex")
                nc.scalar.activation(ex[:], vals[:], AF.Exp)
                sm = Dsml.tile([128, 2], F32, tag="sm")
                nc.vector.reduce_sum(sm[:, :1], ex[:], axis=AX.X)
                nc.vector.reciprocal(sm[:, 1:2], sm[:, :1])
                wk = Dsml.tile([128, top_k], F32, tag="wk")
                nc.vector.tensor_scalar_mul(wk[:], ex[:], sm[:, 1:2])
                nc.sync.dma_start(tki_d.ap()[rsl, :], mi8[:, :top_k])
                nc.sync.dma_start(tkw_d.ap()[rsl, :], wk[:])
                wgt = wgtp.tile([128, D], F32, tag="wgt")
                for j in range(top_k):
                    g = Dstg.tile([128, D], F32, tag="gat")
                    nc.gpsimd.indirect_dma_start(
                        out=g[:], out_offset=None, in_=pool_d.ap(),
                        in_offset=bass.IndirectOffsetOnAxis(ap=mi8[:, j:j + 1], axis=0))
                    if j == 0:
                        nc.vector.tensor_scalar_mul(wgt[:], g[:], wk[:, :1])
                    else:
                        nc.vector.scalar_tensor_tensor(
                            wgt[:], g[:], wk[:, j:j + 1], wgt[:],
                            op0=ALU.mult, op1=ALU.add)

                # --- E: LayerNorm on proj[t]
                st = Dsml.tile([128, 2, 6], F32, tag="st")
                mv = Dsml.tile([128, 2], F32, tag="mv")
                for sg in range(2):
                    nc.vector.bn_stats(out=st[:, sg, :],
                                       in_=proj[:, t, sg * 512:(sg + 1) * 512])
                nc.vector.bn_aggr(out=mv[:], in_=st[:])
                rs = Dsml.tile([128, 1], F32, tag="rs")
                nc.scalar.activation(rs[:], mv[:, 1:2], AF.Sqrt, bias=eps_t[:])
                nc.vector.reciprocal(rs[:], rs[:])
                pln = Dstg.tile([128, D], F32, tag="pln")
                nc.vector.tensor_scalar(pln[:], proj[:, t, :], mv[:, :1], rs[:],
                                        op0=ALU.subtract, op1=ALU.mult)
                if use_gamma:
                    nc.vector.tensor_mul(pln[:], pln[:], gamma_t[:])
                if use_beta:
                    nc.vector.tensor_add(pln[:], pln[:], beta_t[:])

                # --- F: combT = [plnT ; wgtT] (split+transpose), then @ wm
                ch = combp.tile([128, 2 * KI, 128], BF16, tag="ch")
                cl = combp.tile([128, 2 * KI, 128], BF16, name="cl", tag="cl") if split_tail else None
                for half, src in ((0, pln), (1, wgt)):
                    sh = Dstg.tile([128, D], BF16, tag="sh")
                    nc.vector.tensor_copy(sh[:], src[:])
                    sl = None
                    if split_tail:
                        sl = Dstg.tile([128, D], BF16, tag="sl")
                        nc.vector.tensor_sub(sl[:], src[:], sh[:])
                    for c in range(KI):
                        csl = slice(c * 128, (c + 1) * 128)
                        tp = Tps.tile([128, 128], BF16, tag="tp")
                        nc.tensor.transpose(tp[:], sh[:, csl], ident[:])
                        nc.scalar.copy(ch[:, half * KI + c, :], tp[:])
                        if split_tail:
                            tp2 = Tps.tile([128, 128], BF16, tag="tp2")
                            nc.tensor.transpose(tp2[:], sl[:, csl], ident[:])
                            nc.scalar.copy(cl[:, half * KI + c, :], tp2[:])
                for nn in range(2):
                    nsl = slice(nn * 512, (nn + 1) * 512)
                    ps = Fps.tile([128, 512], F32, tag="ps")
                    _mm_acc(nc, ps[:],
                            (lambda c: ch[:, c, :],
                             (lambda c: cl[:, c, :]) if split_tail else None),
                            (lambda c: wmh[:, c, nsl],
                             (lambda c: wml[:, c, nsl]) if split_tail else None),
                            2 * KI, True)
                    ost = Dstg.tile([128, 512], F32, tag="ost")
                    if use_bm:
                        nc.vector.scalar_tensor_tensor(
                            ost[:], ps[:], 1.0, bm_t[:, nsl],
                            op0=ALU.mult, op1=ALU.add)
                    else:
                        nc.scalar.copy(ost[:], ps[:])
                    nc.sync.dma_start(out_d.ap()[rsl, nsl], ost[:])

    nc.finalize()
    return nc


_CACHE = {}
_LAST_RESULTS = None


def _get_program(key, *args):
    if key not in _CACHE:
        _CACHE[key] = _build(*args)
    return _CACHE[key]


def _split_bf16(a):
    hi = a.astype(ml_dtypes.bfloat16)
    lo = (a - hi.astype(np.float32)).astype(ml_dtypes.bfloat16)
    return hi, lo


def kernel(x, tensor_pool, top_k, w1, b1, w2, b2, temperature, wp, bp,
           gamma, beta, wm, bm):
    x = np.asarray(x, dtype=np.float32)
    tensor_pool = np.ascontiguousarray(np.asarray(tensor_pool, np.float32))
    w1 = np.asarray(w1, np.float32); b1 = np.asarray(b1, np.float32)
    w2 = np.asarray(w2, np.float32); b2 = np.asarray(b2, np.float32)
    wp = np.asarray(wp, np.float32); bp = np.asarray(bp, np.float32)
    gamma = np.asarray(gamma, np.float32); beta = np.asarray(beta, np.float32)
    wm = np.asarray(wm, np.float32); bm = np.asarray(bm, np.float32)
    k = int(top_k)
    B, S, _ = x.shape
    ntok = B * S
    temp = float(np.clip(np.float32(temperature), 0.1, 5.0))
    inv_temp = float(np.float32(1.0) / np.float32(temp))

    use_b2 = bool(np.any(b2)); use_bp = bool(np.any(bp))
    use_gamma = not bool(np.all(gamma == 1.0)); use_beta = bool(np.any(beta))
    use_bm = bool(np.any(bm))

    nc = _get_program((inv_temp, k, use_b2, use_bp, use_gamma, use_beta, use_bm),
                      inv_temp, k, use_b2, use_bp, use_gamma, use_beta, use_bm)

    split_router = ROUTER_MODE == "split"
    split_tail = TAIL_MODE == "split"
    w1h, w1l = _split_bf16(w1)
    w2h, w2l = _split_bf16(w2)
    wph, wpl = _split_bf16(wp)
    wmh, wml = _split_bf16(wm)

    xf = x.reshape(ntok, H)
    shard = ntok // NCORES
    in_maps = []
    for c in range(NCORES):
        xs = xf[c * shard:(c + 1) * shard]
        xT = np.ascontiguousarray(xs.T)
        xTh, xTl = _split_bf16(xT)
        m = {"xT_hi": xTh, "w1_hi": w1h, "w2_hi": w2h, "wp_hi": wph,
             "wm_hi": wmh, "b1": b1, "tensor_pool": tensor_pool}
        if split_router or split_tail:
            m["xT_lo"] = xTl
        if split_router:
            m["w1_lo"] = w1l; m["w2_lo"] = w2l
        if split_tail:
            m["wp_lo"] = wpl; m["wm_lo"] = wml
        if use_b2: m["b2"] = b2
        if use_bp: m["bp"] = bp
        if use_gamma: m["gamma"] = gamma
        if use_beta: m["beta"] = beta
        if use_bm: m["bm"] = bm
        in_maps.append(m)

    trace = bool(int(os.environ.get("KERNEL_TRACE", "0")))
    res = run_bass_kernel_spmd(nc, in_maps, core_ids=list(range(NCORES)),
                               trace=trace)
    global _LAST_RESULTS
    _LAST_RESULTS = res

    out = np.concatenate([r["out_sh"] for r in res.results], axis=0)
    idxs = np.concatenate([r["tk_idx"] for r in res.results], axis=0).astype(np.int64)
    ws = np.concatenate([r["tk_w"] for r in res.results], axis=0).astype(np.float32)

    usage = np.zeros((PP,), np.float32)
    np.add.at(usage, idxs.reshape(-1), ws.reshape(-1))
    usage_fraction = usage / (usage.sum(dtype=np.float32) + np.float32(1e-8))
    uniform = np.float32(1.0 / PP)
    scale = min(1.0, x.size / (PP * k))
    diversity_loss = np.float32(
        np.mean(np.square(usage_fraction - uniform)) * scale * 0.01)

    return out.reshape(B, S, D), diversity_loss
